# revision 1
# baseline (speedup 1.0000x reference)
"""Trainium2 Bass kernel for nn_DomainBlock_1520418423078 (GNN message passing).

out[e] = (x[src]+x[dst]) @ w_x + ew[e] @ w_ew_i + (sum_ew[src]+sum_ew[dst]) @ w_ew_j
       = y[src[e]] + y[dst[e]] + ew[e] @ w_ew_i,
  where sum_ew = segment_sum(ew, src),  y = x @ w_x + sum_ew @ w_ew_j.

Two SPMD launches on 8 NeuronCores (edges sharded by src range):
  launch 1: per-core segment_sum via slot-padded sorted stream (DVE tree-add
            within 8-slot blocks + one-hot matmul across blocks) then
            y = [x;sum_ew] @ [w_x;w_ew_j] for the core's nodes.
  host:     assemble y, index y rows into per-edge y[src]/y[dst] streams
            (pure data movement).
  launch 2: stream ew / y[src] / y[dst]; PE computes ew @ w_ew_i via
            transpose + block-diagonal matmul; DVE adds the three terms.
"""

import math
import os

import numpy as np

os.environ.setdefault("NEURON_RT_RESET_CORES", "1")

import concourse.bacc as bacc
import concourse.bass as bass
import concourse.mybir as mybir
import concourse.tile as tile
from concourse import bass_utils

N_CORES = 8
N_NODES = 50000
X_DIM = 32
NODES_PER_CORE = N_NODES // N_CORES          # 6250
N_WIN = 49                                   # 128-node windows per core
TILES_PER_WIN = 5                            # level-1 tiles (128 blocks) per window
WIN_BLK_CAP = TILES_PER_WIN * 128            # 640 blocks per window
NODE_SLOTS = N_WIN * 128                     # 6272 table rows per core
N_L1_TILES = N_WIN * TILES_PER_WIN           # 245
SLOTS_PER_CORE = N_L1_TILES * 1024           # 250880 slot rows
L1_BATCH = 4                                 # level-1 tiles per DMA
EDGE_BATCH = 8192                            # edges per DMA in launch 2
F32 = mybir.dt.float32

_programs = {}


def _build_launch1():
    nc = bacc.Bacc("TRN2", target_bir_lowering=False, debug=False,
                   enable_asserts=False, num_devices=N_CORES)
    d_slots = nc.dram_tensor("slots", [SLOTS_PER_CORE, 32], F32,
                             kind="ExternalInput")
    d_blkT = nc.dram_tensor("blkT", [128, N_L1_TILES], F32,
                            kind="ExternalInput")
    d_xT = nc.dram_tensor("xT", [32, NODE_SLOTS], F32, kind="ExternalInput")
    d_iota = nc.dram_tensor("iota", [128, 128], F32, kind="ExternalInput")
    d_wcat = nc.dram_tensor("wcat", [64, 32], F32, kind="ExternalInput")
    d_y = nc.dram_tensor("y", [NODE_SLOTS, 32], F32, kind="ExternalOutput")

    with tile.TileContext(nc) as tc:
        with tc.tile_pool(name="const", bufs=1) as const, \
             tc.tile_pool(name="sbuf", bufs=3) as sbuf, \
             tc.tile_pool(name="psum", bufs=2, space="PSUM") as psum:
            iota_t = const.tile([128, 128], F32)
            nc.sync.dma_start(iota_t[:], d_iota[:])
            wcat_t = const.tile([64, 32], F32)
            nc.sync.dma_start(wcat_t[:], d_wcat[:])
            blkT_t = const.tile([128, N_L1_TILES], F32)
            nc.sync.dma_start(blkT_t[:], d_blkT[:])
            # stacked: rows 0-31 xT, rows 32-63 sum_ewT (window flushes)
            stacked = const.tile([64, NODE_SLOTS], F32)
            nc.sync.dma_start(stacked[:32, :], d_xT[:])

            slots_v = d_slots[:].rearrange("(t b s) f -> b t (s f)", b=128, s=8)
            n_batches = N_L1_TILES // L1_BATCH + (N_L1_TILES % L1_BATCH != 0)
            batch_tiles = {}
            for bi in range(n_batches):
                t0 = bi * L1_BATCH
                t1 = min(t0 + L1_BATCH, N_L1_TILES)
                bt = sbuf.tile([128, (t1 - t0) * 256], F32, tag="slots")
                nc.sync.dma_start(
                    bt[:].rearrange("b (t sf) -> b t sf", t=t1 - t0),
                    slots_v[:, t0:t1, :])
                batch_tiles[bi] = bt

            for w in range(N_WIN):
                ps = psum.tile([32, 128], F32, space="PSUM", tag="pseg")
                for k in range(TILES_PER_WIN):
                    t = w * TILES_PER_WIN + k
                    bt = batch_tiles[t // L1_BATCH]
                    sl = bt[:, (t % L1_BATCH) * 256:(t % L1_BATCH) * 256 + 256]
                    # tree-add 8 slots -> block sums in sl[:, 0:32]
                    nc.vector.tensor_add(sl[:, 0:128], sl[:, 0:128], sl[:, 128:256])
                    nc.vector.tensor_add(sl[:, 0:64], sl[:, 0:64], sl[:, 64:128])
                    nc.vector.tensor_add(sl[:, 0:32], sl[:, 0:32], sl[:, 32:64])
                    s2 = sbuf.tile([128, 128], F32, tag="s2")
                    nc.vector.tensor_tensor(
                        s2[:], blkT_t[:, t:t + 1].to_broadcast([128, 128]),
                        iota_t[:], mybir.AluOpType.is_equal)
                    nc.tensor.matmul(ps[:], lhsT=sl[:, 0:32], rhs=s2[:],
                                     start=(k == 0), stop=(k == TILES_PER_WIN - 1))
                nc.vector.tensor_copy(stacked[32:64, w * 128:(w + 1) * 128], ps[:])

            # y = stacked.T @ wcat, one 128-node chunk at a time
            for u in range(N_WIN):
                py = psum.tile([128, 32], F32, space="PSUM", tag="py")
                nc.tensor.matmul(py[:], lhsT=stacked[:, u * 128:(u + 1) * 128],
                                 rhs=wcat_t[:], start=True, stop=True)
                yt = sbuf.tile([128, 32], F32, tag="yt")
                nc.vector.tensor_copy(yt[:], py[:])
                nc.sync.dma_start(d_y[u * 128:(u + 1) * 128, :], yt[:])

    nc.compile()
    return nc


def _build_launch2(e_pad):
    nc = bacc.Bacc("TRN2", target_bir_lowering=False, debug=False,
                   enable_asserts=False, num_devices=N_CORES)
    d_ew = nc.dram_tensor("ewb", [e_pad, 32], F32, kind="ExternalInput")
    d_ys = nc.dram_tensor("ysrc", [e_pad, 32], F32, kind="ExternalInput")
    d_yd = nc.dram_tensor("ydst", [e_pad, 32], F32, kind="ExternalInput")
    d_I = nc.dram_tensor("I128", [128, 128], F32, kind="ExternalInput")
    d_W4 = nc.dram_tensor("W4", [128, 128], F32, kind="ExternalInput")
    d_out = nc.dram_tensor("out", [e_pad, 32], F32, kind="ExternalOutput")

    n_batches = e_pad // EDGE_BATCH
    gpb = EDGE_BATCH // 512      # PE groups per batch
    with tile.TileContext(nc) as tc:
        with tc.tile_pool(name="const", bufs=1) as const, \
             tc.tile_pool(name="sbuf", bufs=2) as sbuf, \
             tc.tile_pool(name="psum", bufs=4, space="PSUM") as psum:
            I_t = const.tile([128, 128], F32)
            nc.sync.dma_start(I_t[:], d_I[:])
            W4_t = const.tile([128, 128], F32)
            nc.sync.dma_start(W4_t[:], d_W4[:])
            C = EDGE_BATCH // 128     # 64 rows per partition
            for b in range(n_batches):
                sl = slice(b * EDGE_BATCH, (b + 1) * EDGE_BATCH)
                ewt = sbuf.tile([128, C * 32], F32, tag="ew")
                nc.sync.dma_start(
                    ewt[:], d_ew[sl, :].rearrange("(p c) f -> p (c f)", c=C))
                yst = sbuf.tile([128, C * 32], F32, tag="ys")
                nc.scalar.dma_start(
                    yst[:], d_ys[sl, :].rearrange("(p c) f -> p (c f)", c=C))
                ydt = sbuf.tile([128, C * 32], F32, tag="yd")
                nc.scalar.dma_start(
                    ydt[:], d_yd[sl, :].rearrange("(p c) f -> p (c f)", c=C))
                outt = sbuf.tile([128, C * 32], F32, tag="out")
                for g in range(gpb):
                    gs = slice(g * 128, (g + 1) * 128)
                    pT = psum.tile([128, 128], F32, space="PSUM", tag="pT")
                    nc.tensor.transpose(pT[:], ewt[:, gs], I_t[:])
                    stk = sbuf.tile([128, 128], F32, tag="stk")
                    nc.vector.tensor_copy(stk[:], pT[:])
                    pM = psum.tile([128, 128], F32, space="PSUM", tag="pM")
                    nc.tensor.matmul(pM[:], lhsT=stk[:], rhs=W4_t[:],
                                     start=True, stop=True)
                    nc.vector.tensor_add(outt[:, gs], pM[:], yst[:, gs])
                    nc.vector.tensor_add(outt[:, gs], outt[:, gs], ydt[:, gs])
                nc.sync.dma_start(
                    d_out[sl, :].rearrange("(p c) f -> p (c f)", c=C), outt[:])

    nc.compile()
    return nc


def _host_prep(x, edge_index, edge_weight):
    """Shard edges by src range, build sorted slot streams + metadata."""
    src = np.asarray(edge_index[0])
    dst = np.asarray(edge_index[1])
    ew = np.asarray(edge_weight)
    x = np.asarray(x)

    owner = src // NODES_PER_CORE
    prep = {"cores": []}
    q_glob = np.empty(N_NODES, np.int64)

    for c in range(N_CORES):
        eidx = np.nonzero(owner == c)[0]
        s_loc = src[eidx] - c * NODES_PER_CORE
        order = np.argsort(s_loc, kind="stable")
        sid = eidx[order]                     # edge ids sorted by src
        s_sorted = s_loc[order]
        deg = np.bincount(s_loc, minlength=NODES_PER_CORE)
        blocks = (deg + 7) // 8               # 0 for deg-0 nodes

        # pack nodes into windows: <=128 nodes and <=WIN_BLK_CAP blocks each
        node_order = np.argsort(-blocks, kind="stable")
        win_blocks = np.zeros(N_WIN, np.int64)
        win_nodes = np.zeros(N_WIN, np.int64)
        node_win = np.empty(NODES_PER_CORE, np.int64)
        node_slot = np.empty(NODES_PER_CORE, np.int64)
        for n in node_order:
            b = blocks[n]
            cand = np.nonzero((win_nodes < 128) &
                              (win_blocks + b <= WIN_BLK_CAP))[0]
            assert cand.size > 0, "window packing failed; raise TILES_PER_WIN"
            w = cand[np.argmin(win_blocks[cand])]
            node_win[n] = w
            node_slot[n] = win_nodes[w]
            win_nodes[w] += 1
            win_blocks[w] += b

        q_glob[c * NODES_PER_CORE:(c + 1) * NODES_PER_CORE] = \
            c * NODE_SLOTS + node_win * 128 + node_slot

        # per-window block streams (slot row ids into sid, -1 pad)
        edge_start = np.zeros(NODES_PER_CORE + 1, np.int64)
        np.cumsum(deg, out=edge_start[1:])
        slot_idx = np.full((N_WIN, WIN_BLK_CAP * 8), -1, np.int64)
        blk_rel = np.full((N_WIN, WIN_BLK_CAP), -1, np.int64)
        win_fill = np.zeros(N_WIN, np.int64)
        # nodes in window-slot order for deterministic layout
        for n in np.argsort(node_win * 128 + node_slot, kind="stable"):
            b = blocks[n]
            if b == 0:
                continue
            w = node_win[n]
            f = win_fill[w]
            d = deg[n]
            slot_idx[w, f * 8:f * 8 + d] = np.arange(edge_start[n],
                                                     edge_start[n] + d)
            blk_rel[w, f:f + b] = node_slot[n]
            win_fill[w] += b

        flat = slot_idx.reshape(-1)
        ew_slots = np.zeros((SLOTS_PER_CORE, 32), np.float32)
        valid = flat >= 0
        ew_slots[valid] = ew[sid[flat[valid]]]

        blkT = blk_rel.reshape(N_L1_TILES, 128).T.astype(np.float32).copy()

        xq = np.zeros((NODE_SLOTS, 32), np.float32)
        xq[node_win * 128 + node_slot] = x[c * NODES_PER_CORE:
                                           (c + 1) * NODES_PER_CORE]

        prep["cores"].append({
            "eidx": eidx, "ew_slots": ew_slots, "blkT": blkT,
            "xT": np.ascontiguousarray(xq.T),
        })

    prep["q_glob"] = q_glob
    prep["src"] = src
    prep["dst"] = dst
    return prep


def kernel(x, edge_index, edge_weight, w_x, w_ew_i, w_ew_j):
    x = np.asarray(x, np.float32)
    edge_weight = np.asarray(edge_weight, np.float32)
    w_x = np.asarray(w_x, np.float32)
    w_ew_i = np.asarray(w_ew_i, np.float32)
    w_ew_j = np.asarray(w_ew_j, np.float32)
    E = edge_weight.shape[0]

    prep = _host_prep(x, edge_index, edge_weight)
    iota = np.broadcast_to(np.arange(128, dtype=np.float32),
                           (128, 128)).copy()
    wcat = np.concatenate([w_x, w_ew_j], axis=0)
    I128 = np.eye(128, dtype=np.float32)
    W4 = np.zeros((128, 128), np.float32)
    for cc in range(4):
        W4[cc * 32:(cc + 1) * 32, cc * 32:(cc + 1) * 32] = w_ew_i

    if "l1" not in _programs:
        _programs["l1"] = _build_launch1()
    nc1 = _programs["l1"]
    in1 = [{"slots": pc["ew_slots"], "blkT": pc["blkT"], "xT": pc["xT"],
            "iota": iota, "wcat": wcat} for pc in prep["cores"]]
    res1 = bass_utils.run_bass_kernel_spmd(nc1, in1,
                                           core_ids=list(range(N_CORES)))
    y_q = np.concatenate([res1.results[c]["y"] for c in range(N_CORES)],
                         axis=0)

    qsrc = prep["q_glob"][prep["src"]]
    qdst = prep["q_glob"][prep["dst"]]

    e_pad = max(len(pc["eidx"]) for pc in prep["cores"])
    e_pad = ((e_pad + EDGE_BATCH - 1) // EDGE_BATCH) * EDGE_BATCH
    key = ("l2", e_pad)
    if key not in _programs:
        _programs[key] = _build_launch2(e_pad)
    nc2 = _programs[key]

    in2 = []
    for pc in prep["cores"]:
        eidx = pc["eidx"]
        n = len(eidx)
        ewb = np.zeros((e_pad, 32), np.float32)
        ewb[:n] = edge_weight[eidx]
        ys = np.zeros((e_pad, 32), np.float32)
        ys[:n] = y_q[qsrc[eidx]]
        yd = np.zeros((e_pad, 32), np.float32)
        yd[:n] = y_q[qdst[eidx]]
        in2.append({"ewb": ewb, "ysrc": ys, "ydst": yd, "I128": I128,
                    "W4": W4})
    res2 = bass_utils.run_bass_kernel_spmd(nc2, in2,
                                           core_ids=list(range(N_CORES)))

    out = np.empty((E, 32), np.float32)
    for c in range(N_CORES):
        eidx = prep["cores"][c]["eidx"]
        out[eidx] = res2.results[c]["out"][:len(eidx)]
    return out


# revision 11
# speedup vs baseline: 1.1079x; 1.1079x over previous
"""Trainium2 Bass kernel for nn_DomainBlock_1520418423078 (GNN message passing).

out[e] = (x[src]+x[dst]) @ w_x + ew[e] @ w_ew_i + (sum_ew[src]+sum_ew[dst]) @ w_ew_j
       = y[src[e]] + y[dst[e]] + ew[e] @ w_ew_i,
  where sum_ew = segment_sum(ew, src),  y = x @ w_x + sum_ew @ w_ew_j.

Two SPMD launches on 8 NeuronCores (edges sharded by src range):
  launch 1: per-core segment_sum via slot-padded sorted stream (DVE tree-add
            within 8-slot blocks + one-hot matmul across blocks) then
            y = [x;sum_ew] @ [w_x;w_ew_j] for the core's nodes.
  host:     assemble y, index y rows into per-edge y[src]/y[dst] streams
            (pure data movement).
  launch 2: stream ew / y[src] / y[dst]; PE computes ew @ w_ew_i via
            transpose + block-diagonal matmul; DVE adds the three terms.
"""

import math
import os

import numpy as np

os.environ.setdefault("NEURON_RT_RESET_CORES", "1")

import concourse.bacc as bacc
import concourse.bass as bass
import concourse.mybir as mybir
import concourse.tile as tile
from concourse import bass_utils

N_CORES = 8
N_NODES = 50000
X_DIM = 32
NODES_PER_CORE = N_NODES // N_CORES          # 6250
N_WIN = 49                                   # 128-node windows per core
TILES_PER_WIN = 5                            # level-1 tiles (128 blocks) per window
WIN_BLK_CAP = TILES_PER_WIN * 128            # 640 blocks per window
NODE_SLOTS = N_WIN * 128                     # 6272 table rows per core
N_L1_TILES = N_WIN * TILES_PER_WIN           # 245
SLOTS_PER_CORE = N_L1_TILES * 1024           # 250880 slot rows
L1_BATCH = int(os.environ.get("L1_BATCH", "7"))
L1_MODE = os.environ.get("L1_MODE", "full")  # full | dmaonly | noseg
EDGE_BATCH = int(os.environ.get("EDGE_BATCH", "8192"))
L2_BUFS = int(os.environ.get("L2_BUFS", "2"))
F32 = mybir.dt.float32

_programs = {}


def _build_launch1(reps=1):
    nc = bacc.Bacc("TRN2", target_bir_lowering=False, debug=False,
                   enable_asserts=False, num_devices=N_CORES)
    d_slots = nc.dram_tensor("slots", [128, N_L1_TILES * 256], F32,
                             kind="ExternalInput")
    d_blkT = nc.dram_tensor("blkT", [128, N_L1_TILES], F32,
                            kind="ExternalInput")
    d_xT = nc.dram_tensor("xT", [32, NODE_SLOTS], F32, kind="ExternalInput")
    d_iota = nc.dram_tensor("iota", [128, 128], F32, kind="ExternalInput")
    d_wcat = nc.dram_tensor("wcat", [64, 32], F32, kind="ExternalInput")
    d_y = nc.dram_tensor("y", [NODE_SLOTS, 32], F32, kind="ExternalOutput")

    with tile.TileContext(nc) as tc:
        with tc.tile_pool(name="const", bufs=1) as const, \
             tc.tile_pool(name="sbuf", bufs=3) as sbuf, \
             tc.tile_pool(name="psum", bufs=4, space="PSUM") as psum:
            iota_t = const.tile([128, 128], F32)
            nc.sync.dma_start(iota_t[:], d_iota[:])
            wcat_t = const.tile([64, 32], F32)
            nc.sync.dma_start(wcat_t[:], d_wcat[:])
            blkT_t = const.tile([128, N_L1_TILES], F32)
            nc.sync.dma_start(blkT_t[:], d_blkT[:])
            # stacked: rows 0-31 xT, rows 32-63 sum_ewT (window flushes)
            stacked = const.tile([64, NODE_SLOTS], F32)

            import contextlib
            loop_cm = tc.For_i(0, reps, 1) if reps > 1 else contextlib.nullcontext()
            with loop_cm:
                nc.sync.dma_start(stacked[:32, :], d_xT[:])
                _launch1_body(nc, tc, sbuf, psum, d_slots, d_y, blkT_t, iota_t,
                              wcat_t, stacked)

    nc.compile()
    return nc


def _launch1_body(nc, tc, sbuf, psum, d_slots, d_y, blkT_t, iota_t, wcat_t,
                  stacked):
            n_batches = N_L1_TILES // L1_BATCH + (N_L1_TILES % L1_BATCH != 0)
            batch_tiles = {}
            s2_tiles = {}
            for bi in range(n_batches):
                t0 = bi * L1_BATCH
                t1 = min(t0 + L1_BATCH, N_L1_TILES)
                nt = t1 - t0
                bt = sbuf.tile([128, nt * 256], F32, tag="slots")
                nc.sync.dma_start(bt[:], d_slots[:, t0 * 256:t1 * 256])
                batch_tiles[bi] = bt
                # batched tree-add: 8 slots -> block sums at [:, t, 0:32]
                btv = bt[:].rearrange("b (t sf) -> b t sf", t=nt)
                if L1_MODE == "dmaonly":
                    continue
                nc.gpsimd.tensor_tensor(btv[:, :, 0:128], btv[:, :, 0:128],
                                        btv[:, :, 128:256],
                                        mybir.AluOpType.add)
                nc.vector.tensor_tensor(btv[:, :, 0:64], btv[:, :, 0:64],
                                        btv[:, :, 64:128], mybir.AluOpType.add)
                nc.vector.tensor_tensor(btv[:, :, 0:32], btv[:, :, 0:32],
                                        btv[:, :, 32:64], mybir.AluOpType.add)
                # batched one-hot build for nt tiles
                s2 = sbuf.tile([128, nt * 128], F32, tag="s2")
                nc.vector.tensor_tensor(
                    s2[:].rearrange("p (t f) -> p t f", t=nt),
                    blkT_t[:, t0:t1].rearrange("p (t o) -> p t o", o=1).to_broadcast(
                        [128, nt, 128]),
                    iota_t[:].rearrange("p (o f) -> p o f", o=1).to_broadcast(
                        [128, nt, 128]),
                    mybir.AluOpType.is_equal)
                s2_tiles[bi] = s2

            for w in range(N_WIN if L1_MODE == "full" else 0):
                ps = psum.tile([32, 128], F32, space="PSUM", tag="pseg")
                for k in range(TILES_PER_WIN):
                    t = w * TILES_PER_WIN + k
                    bt = batch_tiles[t // L1_BATCH]
                    s2 = s2_tiles[t // L1_BATCH]
                    j = t % L1_BATCH
                    nc.tensor.matmul(ps[:], lhsT=bt[:, j * 256:j * 256 + 32],
                                     rhs=s2[:, j * 128:(j + 1) * 128],
                                     start=(k == 0), stop=(k == TILES_PER_WIN - 1))
                nc.scalar.copy(stacked[32:64, w * 128:(w + 1) * 128], ps[:])

            # y = stacked.T @ wcat, one 128-node chunk at a time
            for u in range(N_WIN if L1_MODE == "full" else 1):
                py = psum.tile([128, 32], F32, space="PSUM", tag="py")
                nc.tensor.matmul(py[:], lhsT=stacked[:, u * 128:(u + 1) * 128],
                                 rhs=wcat_t[:], start=True, stop=True)
                yt = sbuf.tile([128, 32], F32, tag="yt")
                nc.vector.tensor_copy(yt[:], py[:])
                nc.sync.dma_start(d_y[u * 128:(u + 1) * 128, :], yt[:])


def _build_launch2(e_pad, reps=1):
    nc = bacc.Bacc("TRN2", target_bir_lowering=False, debug=False,
                   enable_asserts=False, num_devices=N_CORES)
    d_ew = nc.dram_tensor("ewb", [e_pad, 32], F32, kind="ExternalInput")
    d_ys = nc.dram_tensor("ysrc", [e_pad, 32], F32, kind="ExternalInput")
    d_yd = nc.dram_tensor("ydst", [e_pad, 32], F32, kind="ExternalInput")
    d_I = nc.dram_tensor("I128", [128, 128], F32, kind="ExternalInput")
    d_W4 = nc.dram_tensor("W4", [128, 128], F32, kind="ExternalInput")
    d_out = nc.dram_tensor("out", [e_pad, 32], F32, kind="ExternalOutput")

    n_batches = e_pad // EDGE_BATCH
    gpb = EDGE_BATCH // 512      # PE groups per batch
    with tile.TileContext(nc) as tc:
        with tc.tile_pool(name="const", bufs=1) as const, \
             tc.tile_pool(name="sbuf", bufs=L2_BUFS) as sbuf, \
             tc.tile_pool(name="psum", bufs=4, space="PSUM") as psum:
            I_t = const.tile([128, 128], F32)
            nc.sync.dma_start(I_t[:], d_I[:])
            W4_t = const.tile([128, 128], F32)
            nc.sync.dma_start(W4_t[:], d_W4[:])
            C = EDGE_BATCH // 128     # 64 rows per partition
            import contextlib
            loop_cm = tc.For_i(0, reps, 1) if reps > 1 else contextlib.nullcontext()
            with loop_cm:
                _launch2_body(nc, tc, sbuf, psum, d_ew, d_ys, d_yd, d_out,
                              I_t, W4_t, n_batches, gpb, C)

    nc.compile()
    return nc


def _launch2_body(nc, tc, sbuf, psum, d_ew, d_ys, d_yd, d_out, I_t, W4_t,
                  n_batches, gpb, C):
            for b in range(n_batches):
                sl = slice(b * EDGE_BATCH, (b + 1) * EDGE_BATCH)
                ewt = sbuf.tile([128, C * 32], F32, tag="ew")
                nc.sync.dma_start(
                    ewt[:], d_ew[sl, :].rearrange("(p c) f -> p (c f)", c=C))
                yst = sbuf.tile([128, C * 32], F32, tag="ys")
                nc.scalar.dma_start(
                    yst[:], d_ys[sl, :].rearrange("(p c) f -> p (c f)", c=C))
                ydt = sbuf.tile([128, C * 32], F32, tag="yd")
                nc.scalar.dma_start(
                    ydt[:], d_yd[sl, :].rearrange("(p c) f -> p (c f)", c=C))
                outt = sbuf.tile([128, C * 32], F32, tag="out")
                # y[src]+y[dst] in one batched add on the (otherwise idle) Pool
                nc.gpsimd.tensor_tensor(yst[:], yst[:], ydt[:],
                                        mybir.AluOpType.add)
                for g in range(gpb):
                    gs = slice(g * 128, (g + 1) * 128)
                    pT = psum.tile([128, 128], F32, space="PSUM", tag="pT")
                    nc.tensor.transpose(pT[:], ewt[:, gs], I_t[:])
                    stk = sbuf.tile([128, 128], F32, tag="stk")
                    nc.scalar.copy(stk[:], pT[:])
                    pM = psum.tile([128, 128], F32, space="PSUM", tag="pM")
                    nc.tensor.matmul(pM[:], lhsT=stk[:], rhs=W4_t[:],
                                     start=True, stop=True)
                    nc.vector.tensor_add(outt[:, gs], pM[:], yst[:, gs])
                nc.sync.dma_start(
                    d_out[sl, :].rearrange("(p c) f -> p (c f)", c=C), outt[:])


def _host_prep(x, edge_index, edge_weight):
    """Shard edges by src range, build sorted slot streams + metadata."""
    src = np.asarray(edge_index[0])
    dst = np.asarray(edge_index[1])
    ew = np.asarray(edge_weight)
    x = np.asarray(x)

    owner = src // NODES_PER_CORE
    prep = {"cores": []}
    q_glob = np.empty(N_NODES, np.int64)

    for c in range(N_CORES):
        eidx = np.nonzero(owner == c)[0]
        s_loc = src[eidx] - c * NODES_PER_CORE
        order = np.argsort(s_loc, kind="stable")
        sid = eidx[order]                     # edge ids sorted by src
        s_sorted = s_loc[order]
        deg = np.bincount(s_loc, minlength=NODES_PER_CORE)
        blocks = (deg + 7) // 8               # 0 for deg-0 nodes

        # pack nodes into windows: <=128 nodes and <=WIN_BLK_CAP blocks each
        node_order = np.argsort(-blocks, kind="stable")
        win_blocks = np.zeros(N_WIN, np.int64)
        win_nodes = np.zeros(N_WIN, np.int64)
        node_win = np.empty(NODES_PER_CORE, np.int64)
        node_slot = np.empty(NODES_PER_CORE, np.int64)
        for n in node_order:
            b = blocks[n]
            cand = np.nonzero((win_nodes < 128) &
                              (win_blocks + b <= WIN_BLK_CAP))[0]
            assert cand.size > 0, "window packing failed; raise TILES_PER_WIN"
            w = cand[np.argmin(win_blocks[cand])]
            node_win[n] = w
            node_slot[n] = win_nodes[w]
            win_nodes[w] += 1
            win_blocks[w] += b

        q_glob[c * NODES_PER_CORE:(c + 1) * NODES_PER_CORE] = \
            c * NODE_SLOTS + node_win * 128 + node_slot

        # per-window block streams (slot row ids into sid, -1 pad)
        edge_start = np.zeros(NODES_PER_CORE + 1, np.int64)
        np.cumsum(deg, out=edge_start[1:])
        slot_idx = np.full((N_WIN, WIN_BLK_CAP * 8), -1, np.int64)
        blk_rel = np.full((N_WIN, WIN_BLK_CAP), -1, np.int64)
        win_fill = np.zeros(N_WIN, np.int64)
        # nodes in window-slot order for deterministic layout
        for n in np.argsort(node_win * 128 + node_slot, kind="stable"):
            b = blocks[n]
            if b == 0:
                continue
            w = node_win[n]
            f = win_fill[w]
            d = deg[n]
            slot_idx[w, f * 8:f * 8 + d] = np.arange(edge_start[n],
                                                     edge_start[n] + d)
            blk_rel[w, f:f + b] = node_slot[n]
            win_fill[w] += b

        # transpose to [128, tiles*8] so device loads are per-partition
        # contiguous: slotsH[p, (t, s, f)] = slot (t*128+p)*8+s
        flat = slot_idx.reshape(N_L1_TILES, 128, 8).transpose(1, 0, 2).reshape(-1)
        ew_slots = np.zeros((flat.size, 32), np.float32)
        valid = flat >= 0
        ew_slots[valid] = ew[sid[flat[valid]]]
        ew_slots = ew_slots.reshape(128, N_L1_TILES * 256)

        blkT = blk_rel.reshape(N_L1_TILES, 128).T.astype(np.float32).copy()

        xq = np.zeros((NODE_SLOTS, 32), np.float32)
        xq[node_win * 128 + node_slot] = x[c * NODES_PER_CORE:
                                           (c + 1) * NODES_PER_CORE]

        prep["cores"].append({
            "eidx": eidx, "ew_slots": ew_slots, "blkT": blkT,
            "xT": np.ascontiguousarray(xq.T),
        })

    prep["q_glob"] = q_glob
    prep["src"] = src
    prep["dst"] = dst
    return prep


def kernel(x, edge_index, edge_weight, w_x, w_ew_i, w_ew_j):
    x = np.asarray(x, np.float32)
    edge_weight = np.asarray(edge_weight, np.float32)
    w_x = np.asarray(w_x, np.float32)
    w_ew_i = np.asarray(w_ew_i, np.float32)
    w_ew_j = np.asarray(w_ew_j, np.float32)
    E = edge_weight.shape[0]

    prep = _host_prep(x, edge_index, edge_weight)
    iota = np.broadcast_to(np.arange(128, dtype=np.float32),
                           (128, 128)).copy()
    wcat = np.concatenate([w_x, w_ew_j], axis=0)
    I128 = np.eye(128, dtype=np.float32)
    W4 = np.zeros((128, 128), np.float32)
    for cc in range(4):
        W4[cc * 32:(cc + 1) * 32, cc * 32:(cc + 1) * 32] = w_ew_i

    if "l1" not in _programs:
        _programs["l1"] = _build_launch1()
    nc1 = _programs["l1"]
    in1 = [{"slots": pc["ew_slots"], "blkT": pc["blkT"], "xT": pc["xT"],
            "iota": iota, "wcat": wcat} for pc in prep["cores"]]
    res1 = bass_utils.run_bass_kernel_spmd(nc1, in1,
                                           core_ids=list(range(N_CORES)))
    y_q = np.concatenate([res1.results[c]["y"] for c in range(N_CORES)],
                         axis=0)

    qsrc = prep["q_glob"][prep["src"]]
    qdst = prep["q_glob"][prep["dst"]]

    e_pad = max(len(pc["eidx"]) for pc in prep["cores"])
    e_pad = ((e_pad + EDGE_BATCH - 1) // EDGE_BATCH) * EDGE_BATCH
    key = ("l2", e_pad)
    if key not in _programs:
        _programs[key] = _build_launch2(e_pad)
    nc2 = _programs[key]

    in2 = []
    for pc in prep["cores"]:
        eidx = pc["eidx"]
        n = len(eidx)
        ewb = np.zeros((e_pad, 32), np.float32)
        ewb[:n] = edge_weight[eidx]
        ys = np.zeros((e_pad, 32), np.float32)
        ys[:n] = y_q[qsrc[eidx]]
        yd = np.zeros((e_pad, 32), np.float32)
        yd[:n] = y_q[qdst[eidx]]
        in2.append({"ewb": ewb, "ysrc": ys, "ydst": yd, "I128": I128,
                    "W4": W4})
    res2 = bass_utils.run_bass_kernel_spmd(nc2, in2,
                                           core_ids=list(range(N_CORES)))

    out = np.empty((E, 32), np.float32)
    for c in range(N_CORES):
        eidx = prep["cores"][c]["eidx"]
        out[eidx] = res2.results[c]["out"][:len(eidx)]
    return out


# revision 12
# speedup vs baseline: 41169.2501x; 37159.6457x over previous
"""Trainium2 Bass kernel for nn_DomainBlock_1520418423078 (GNN message passing).

out[e] = (x[src]+x[dst]) @ w_x + ew[e] @ w_ew_i + (sum_ew[src]+sum_ew[dst]) @ w_ew_j
       = y[src[e]] + y[dst[e]] + ew[e] @ w_ew_i,
  where sum_ew = segment_sum(ew, src),  y = x @ w_x + sum_ew @ w_ew_j.

Two SPMD launches on 8 NeuronCores (edges sharded by src range):
  launch 1: per-core segment_sum via slot-padded sorted stream (DVE tree-add
            within 8-slot blocks + one-hot matmul across blocks) then
            y = [x;sum_ew] @ [w_x;w_ew_j] for the core's nodes.
  host:     assemble y, index y rows into per-edge y[src]/y[dst] streams
            (pure data movement).
  launch 2: stream ew / y[src] / y[dst]; PE computes ew @ w_ew_i via
            transpose + block-diagonal matmul; DVE adds the three terms.
"""

import math
import os

import numpy as np

os.environ.setdefault("NEURON_RT_RESET_CORES", "1")

import concourse.bacc as bacc
import concourse.bass as bass
import concourse.mybir as mybir
import concourse.tile as tile
from concourse import bass_utils

N_CORES = 8
N_NODES = 50000
X_DIM = 32
NODES_PER_CORE = N_NODES // N_CORES          # 6250
N_WIN = 49                                   # 128-node windows per core
TILES_PER_WIN = 5                            # level-1 tiles (128 blocks) per window
WIN_BLK_CAP = TILES_PER_WIN * 128            # 640 blocks per window
NODE_SLOTS = N_WIN * 128                     # 6272 table rows per core
N_L1_TILES = N_WIN * TILES_PER_WIN           # 245
SLOTS_PER_CORE = N_L1_TILES * 1024           # 250880 slot rows
L1_BATCH = int(os.environ.get("L1_BATCH", "7"))
L1_MODE = os.environ.get("L1_MODE", "full")  # full | dmaonly | noseg
EDGE_BATCH = int(os.environ.get("EDGE_BATCH", "8192"))
L2_BUFS = int(os.environ.get("L2_BUFS", "2"))
F32 = mybir.dt.float32

_programs = {}


def _build_launch1(reps=1):
    nc = bacc.Bacc("TRN2", target_bir_lowering=False, debug=False,
                   enable_asserts=False, num_devices=N_CORES)
    d_slots = nc.dram_tensor("slots", [128, N_L1_TILES * 256], F32,
                             kind="ExternalInput")
    d_blkT = nc.dram_tensor("blkT", [128, N_L1_TILES], F32,
                            kind="ExternalInput")
    d_xT = nc.dram_tensor("xT", [32, NODE_SLOTS], F32, kind="ExternalInput")
    d_iota = nc.dram_tensor("iota", [128, 128], F32, kind="ExternalInput")
    d_wcat = nc.dram_tensor("wcat", [64, 32], F32, kind="ExternalInput")
    d_y = nc.dram_tensor("y", [NODE_SLOTS, 32], F32, kind="ExternalOutput")

    with tile.TileContext(nc) as tc:
        with tc.tile_pool(name="const", bufs=1) as const, \
             tc.tile_pool(name="sbuf", bufs=3) as sbuf, \
             tc.tile_pool(name="psum", bufs=4, space="PSUM") as psum:
            iota_t = const.tile([128, 128], F32)
            nc.sync.dma_start(iota_t[:], d_iota[:])
            wcat_t = const.tile([64, 32], F32)
            nc.sync.dma_start(wcat_t[:], d_wcat[:])
            blkT_t = const.tile([128, N_L1_TILES], F32)
            nc.sync.dma_start(blkT_t[:], d_blkT[:])
            # stacked: rows 0-31 xT, rows 32-63 sum_ewT (window flushes)
            stacked = const.tile([64, NODE_SLOTS], F32)

            import contextlib
            loop_cm = tc.For_i(0, reps, 1) if reps > 1 else contextlib.nullcontext()
            with loop_cm:
                nc.sync.dma_start(stacked[:32, :], d_xT[:])
                _launch1_body(nc, tc, sbuf, psum, d_slots, d_y, blkT_t, iota_t,
                              wcat_t, stacked)

    nc.compile()
    return nc


def _launch1_body(nc, tc, sbuf, psum, d_slots, d_y, blkT_t, iota_t, wcat_t,
                  stacked):
            n_batches = N_L1_TILES // L1_BATCH + (N_L1_TILES % L1_BATCH != 0)
            batch_tiles = {}
            s2_tiles = {}
            for bi in range(n_batches):
                t0 = bi * L1_BATCH
                t1 = min(t0 + L1_BATCH, N_L1_TILES)
                nt = t1 - t0
                bt = sbuf.tile([128, nt * 256], F32, tag="slots")
                nc.sync.dma_start(bt[:], d_slots[:, t0 * 256:t1 * 256])
                batch_tiles[bi] = bt
                # batched tree-add: 8 slots -> block sums at [:, t, 0:32]
                btv = bt[:].rearrange("b (t sf) -> b t sf", t=nt)
                if L1_MODE == "dmaonly":
                    continue
                nc.gpsimd.tensor_tensor(btv[:, :, 0:128], btv[:, :, 0:128],
                                        btv[:, :, 128:256],
                                        mybir.AluOpType.add)
                nc.vector.tensor_tensor(btv[:, :, 0:64], btv[:, :, 0:64],
                                        btv[:, :, 64:128], mybir.AluOpType.add)
                nc.vector.tensor_tensor(btv[:, :, 0:32], btv[:, :, 0:32],
                                        btv[:, :, 32:64], mybir.AluOpType.add)
                # batched one-hot build for nt tiles
                s2 = sbuf.tile([128, nt * 128], F32, tag="s2")
                nc.vector.tensor_tensor(
                    s2[:].rearrange("p (t f) -> p t f", t=nt),
                    blkT_t[:, t0:t1].rearrange("p (t o) -> p t o", o=1).to_broadcast(
                        [128, nt, 128]),
                    iota_t[:].rearrange("p (o f) -> p o f", o=1).to_broadcast(
                        [128, nt, 128]),
                    mybir.AluOpType.is_equal)
                s2_tiles[bi] = s2

            for w in range(N_WIN if L1_MODE == "full" else 0):
                ps = psum.tile([32, 128], F32, space="PSUM", tag="pseg")
                for k in range(TILES_PER_WIN):
                    t = w * TILES_PER_WIN + k
                    bt = batch_tiles[t // L1_BATCH]
                    s2 = s2_tiles[t // L1_BATCH]
                    j = t % L1_BATCH
                    nc.tensor.matmul(ps[:], lhsT=bt[:, j * 256:j * 256 + 32],
                                     rhs=s2[:, j * 128:(j + 1) * 128],
                                     start=(k == 0), stop=(k == TILES_PER_WIN - 1))
                nc.scalar.copy(stacked[32:64, w * 128:(w + 1) * 128], ps[:])

            # y = stacked.T @ wcat, one 128-node chunk at a time
            for u in range(N_WIN if L1_MODE == "full" else 1):
                py = psum.tile([128, 32], F32, space="PSUM", tag="py")
                nc.tensor.matmul(py[:], lhsT=stacked[:, u * 128:(u + 1) * 128],
                                 rhs=wcat_t[:], start=True, stop=True)
                yt = sbuf.tile([128, 32], F32, tag="yt")
                nc.vector.tensor_copy(yt[:], py[:])
                nc.sync.dma_start(d_y[u * 128:(u + 1) * 128, :], yt[:])


def _build_launch2(e_pad, reps=1):
    nc = bacc.Bacc("TRN2", target_bir_lowering=False, debug=False,
                   enable_asserts=False, num_devices=N_CORES)
    d_ew = nc.dram_tensor("ewb", [e_pad, 32], F32, kind="ExternalInput")
    d_ys = nc.dram_tensor("ysrc", [e_pad, 32], F32, kind="ExternalInput")
    d_yd = nc.dram_tensor("ydst", [e_pad, 32], F32, kind="ExternalInput")
    d_I = nc.dram_tensor("I128", [128, 128], F32, kind="ExternalInput")
    d_W4 = nc.dram_tensor("W4", [128, 128], F32, kind="ExternalInput")
    d_out = nc.dram_tensor("out", [e_pad, 32], F32, kind="ExternalOutput")

    n_batches = e_pad // EDGE_BATCH
    gpb = EDGE_BATCH // 512      # PE groups per batch
    with tile.TileContext(nc) as tc:
        with tc.tile_pool(name="const", bufs=1) as const, \
             tc.tile_pool(name="sbuf", bufs=L2_BUFS) as sbuf, \
             tc.tile_pool(name="psum", bufs=4, space="PSUM") as psum:
            I_t = const.tile([128, 128], F32)
            nc.sync.dma_start(I_t[:], d_I[:])
            W4_t = const.tile([128, 128], F32)
            nc.sync.dma_start(W4_t[:], d_W4[:])
            C = EDGE_BATCH // 128     # 64 rows per partition
            import contextlib
            loop_cm = tc.For_i(0, reps, 1) if reps > 1 else contextlib.nullcontext()
            with loop_cm:
                _launch2_body(nc, tc, sbuf, psum, d_ew, d_ys, d_yd, d_out,
                              I_t, W4_t, n_batches, gpb, C)

    nc.compile()
    return nc


def _launch2_body(nc, tc, sbuf, psum, d_ew, d_ys, d_yd, d_out, I_t, W4_t,
                  n_batches, gpb, C):
            for b in range(n_batches):
                sl = slice(b * EDGE_BATCH, (b + 1) * EDGE_BATCH)
                ewt = sbuf.tile([128, C * 32], F32, tag="ew")
                nc.sync.dma_start(
                    ewt[:], d_ew[sl, :].rearrange("(p c) f -> p (c f)", c=C))
                yst = sbuf.tile([128, C * 32], F32, tag="ys")
                nc.scalar.dma_start(
                    yst[:], d_ys[sl, :].rearrange("(p c) f -> p (c f)", c=C))
                ydt = sbuf.tile([128, C * 32], F32, tag="yd")
                nc.scalar.dma_start(
                    ydt[:], d_yd[sl, :].rearrange("(p c) f -> p (c f)", c=C))
                outt = sbuf.tile([128, C * 32], F32, tag="out")
                # y[src]+y[dst] in one batched add on the (otherwise idle) Pool
                nc.gpsimd.tensor_tensor(yst[:], yst[:], ydt[:],
                                        mybir.AluOpType.add)
                for g in range(gpb):
                    gs = slice(g * 128, (g + 1) * 128)
                    pT = psum.tile([128, 128], F32, space="PSUM", tag="pT")
                    nc.tensor.transpose(pT[:], ewt[:, gs], I_t[:])
                    stk = sbuf.tile([128, 128], F32, tag="stk")
                    nc.scalar.copy(stk[:], pT[:])
                    pM = psum.tile([128, 128], F32, space="PSUM", tag="pM")
                    nc.tensor.matmul(pM[:], lhsT=stk[:], rhs=W4_t[:],
                                     start=True, stop=True)
                    nc.vector.tensor_add(outt[:, gs], pM[:], yst[:, gs])
                nc.sync.dma_start(
                    d_out[sl, :].rearrange("(p c) f -> p (c f)", c=C), outt[:])


def _host_prep(x, edge_index, edge_weight):
    """Shard edges by src range, build sorted slot streams + metadata."""
    src = np.asarray(edge_index[0])
    dst = np.asarray(edge_index[1])
    ew = np.asarray(edge_weight)
    x = np.asarray(x)

    owner = src // NODES_PER_CORE
    prep = {"cores": []}
    q_glob = np.empty(N_NODES, np.int64)

    for c in range(N_CORES):
        eidx = np.nonzero(owner == c)[0]
        s_loc = src[eidx] - c * NODES_PER_CORE
        order = np.argsort(s_loc, kind="stable")
        sid = eidx[order]                     # edge ids sorted by src
        s_sorted = s_loc[order]
        deg = np.bincount(s_loc, minlength=NODES_PER_CORE)
        blocks = (deg + 7) // 8               # 0 for deg-0 nodes

        # pack nodes into windows (<=128 nodes, <=WIN_BLK_CAP blocks each):
        # cyclic assignment in descending-block order balances block load
        node_order = np.argsort(-blocks, kind="stable")
        rank = np.empty(NODES_PER_CORE, np.int64)
        rank[node_order] = np.arange(NODES_PER_CORE)
        node_win = rank % N_WIN
        node_slot = rank // N_WIN
        win_blocks = np.bincount(node_win, weights=blocks,
                                 minlength=N_WIN).astype(np.int64)
        assert win_blocks.max() <= WIN_BLK_CAP, \
            "window packing overflow; raise TILES_PER_WIN"

        q_glob[c * NODES_PER_CORE:(c + 1) * NODES_PER_CORE] = \
            c * NODE_SLOTS + node_win * 128 + node_slot

        # per-window block streams (slot row ids into sid, -1 pad),
        # nodes laid out window-major in (win, slot) order
        edge_start = np.zeros(NODES_PER_CORE + 1, np.int64)
        np.cumsum(deg, out=edge_start[1:])
        slot_idx = np.full(N_WIN * WIN_BLK_CAP * 8, -1, np.int64)
        blk_rel = np.full(N_WIN * WIN_BLK_CAP, -1, np.int64)
        perm = np.argsort(node_win * 128 + node_slot, kind="stable")
        blk_p = blocks[perm]
        deg_p = deg[perm]
        win_p = node_win[perm]
        cum = np.cumsum(blk_p) - blk_p           # global block prefix
        win_base = np.zeros(N_WIN, np.int64)
        np.cumsum(win_blocks[:-1], out=win_base[1:])
        off = cum - win_base[win_p]              # block offset within window
        blk_start = win_p * WIN_BLK_CAP + off    # node's first block pos
        # blk_rel fill: node's blocks get its slot id
        tb = int(blk_p.sum())
        r_blk = np.arange(tb) - np.repeat(np.cumsum(blk_p) - blk_p, blk_p)
        blk_rel[np.repeat(blk_start, blk_p) + r_blk] = \
            np.repeat(node_slot[perm], blk_p)
        # slot_idx fill: node's edges (rows of sorted stream) placed at
        # slot positions blk_start*8 ..
        te = int(deg_p.sum())
        r_e = np.arange(te) - np.repeat(np.cumsum(deg_p) - deg_p, deg_p)
        slot_idx[np.repeat(blk_start * 8, deg_p) + r_e] = \
            np.repeat(edge_start[perm], deg_p) + r_e
        slot_idx = slot_idx.reshape(N_WIN, WIN_BLK_CAP * 8)
        blk_rel = blk_rel.reshape(N_WIN, WIN_BLK_CAP)

        # transpose to [128, tiles*8] so device loads are per-partition
        # contiguous: slotsH[p, (t, s, f)] = slot (t*128+p)*8+s
        flat = slot_idx.reshape(N_L1_TILES, 128, 8).transpose(1, 0, 2).reshape(-1)
        ew_slots = np.zeros((flat.size, 32), np.float32)
        valid = flat >= 0
        ew_slots[valid] = ew[sid[flat[valid]]]
        ew_slots = ew_slots.reshape(128, N_L1_TILES * 256)

        blkT = blk_rel.reshape(N_L1_TILES, 128).T.astype(np.float32).copy()

        xq = np.zeros((NODE_SLOTS, 32), np.float32)
        xq[node_win * 128 + node_slot] = x[c * NODES_PER_CORE:
                                           (c + 1) * NODES_PER_CORE]

        prep["cores"].append({
            "eidx": eidx, "ew_slots": ew_slots, "blkT": blkT,
            "xT": np.ascontiguousarray(xq.T),
        })

    prep["q_glob"] = q_glob
    prep["src"] = src
    prep["dst"] = dst
    return prep


def kernel(x, edge_index, edge_weight, w_x, w_ew_i, w_ew_j):
    x = np.asarray(x, np.float32)
    edge_weight = np.asarray(edge_weight, np.float32)
    w_x = np.asarray(w_x, np.float32)
    w_ew_i = np.asarray(w_ew_i, np.float32)
    w_ew_j = np.asarray(w_ew_j, np.float32)
    E = edge_weight.shape[0]

    prep = _host_prep(x, edge_index, edge_weight)
    iota = np.broadcast_to(np.arange(128, dtype=np.float32),
                           (128, 128)).copy()
    wcat = np.concatenate([w_x, w_ew_j], axis=0)
    I128 = np.eye(128, dtype=np.float32)
    W4 = np.zeros((128, 128), np.float32)
    for cc in range(4):
        W4[cc * 32:(cc + 1) * 32, cc * 32:(cc + 1) * 32] = w_ew_i

    if "l1" not in _programs:
        _programs["l1"] = _build_launch1()
    nc1 = _programs["l1"]
    in1 = [{"slots": pc["ew_slots"], "blkT": pc["blkT"], "xT": pc["xT"],
            "iota": iota, "wcat": wcat} for pc in prep["cores"]]
    res1 = bass_utils.run_bass_kernel_spmd(nc1, in1,
                                           core_ids=list(range(N_CORES)))
    y_q = np.concatenate([res1.results[c]["y"] for c in range(N_CORES)],
                         axis=0)

    qsrc = prep["q_glob"][prep["src"]]
    qdst = prep["q_glob"][prep["dst"]]

    e_pad = max(len(pc["eidx"]) for pc in prep["cores"])
    e_pad = ((e_pad + EDGE_BATCH - 1) // EDGE_BATCH) * EDGE_BATCH
    key = ("l2", e_pad)
    if key not in _programs:
        _programs[key] = _build_launch2(e_pad)
    nc2 = _programs[key]

    in2 = []
    for pc in prep["cores"]:
        eidx = pc["eidx"]
        n = len(eidx)
        ewb = np.zeros((e_pad, 32), np.float32)
        ewb[:n] = edge_weight[eidx]
        ys = np.zeros((e_pad, 32), np.float32)
        ys[:n] = y_q[qsrc[eidx]]
        yd = np.zeros((e_pad, 32), np.float32)
        yd[:n] = y_q[qdst[eidx]]
        in2.append({"ewb": ewb, "ysrc": ys, "ydst": yd, "I128": I128,
                    "W4": W4})
    res2 = bass_utils.run_bass_kernel_spmd(nc2, in2,
                                           core_ids=list(range(N_CORES)))

    out = np.empty((E, 32), np.float32)
    for c in range(N_CORES):
        eidx = prep["cores"][c]["eidx"]
        out[eidx] = res2.results[c]["out"][:len(eidx)]
    return out
